# revision 33
# baseline (speedup 1.0000x reference)
"""MinGRU (parallel log-space scan) Trainium2 Bass kernel.

Problem (hardcoded):
    x:    [B=8, S=4096, D=1024] f32
    W_hg: [D=1024, 2*D=2048]    f32
    out:  [B=8, S=4096, D=1024] f32

    hg = x @ W_hg ; hidden, gate = split(hg)
    h_t = (1-z_t) * h_{t-1} + z_t * g(hidden_t),  z = sigmoid(gate),
    g(v) = v + 0.5 if v >= 0 else sigmoid(v)  ==  max(v + 0.5, sigmoid(v))

Sharding: data-parallel over batch, one batch row per NeuronCore (8 cores),
W_hg replicated.

Layout strategy: the scan must run along the free dimension (channels on
partitions), so the device works entirely in the transposed layout
hg^T/h^T = [channels, seq]. The host passes x pre-transposed per batch row
and transposes the returned h^T back, so the device does no layout
conversion at all — the PE runs only the projection matmuls (fp32r,
full rate), ACT runs the sigmoids, and the DVE runs the fused pointwise
ops plus the native tensor_tensor_scan linear recurrence.

Per-core pipeline over seq chunks of C=512:
  DMA x^T chunk tiles [128d, C]
  -> fp32r matmuls hg^T[k] = sum_j W[j,k]^T x^T[j] accumulated in PSUM
  -> ACT: a = sigmoid(-gate), sigh = sigmoid(hidden)      [PSUM -> SBUF]
  -> DVE: gh = (hidden + 0.5) max sigh ; bneg = (a - 1) * gh
  -> DVE: h = scan(a * h_prev) - bneg   (carry chained across chunks)
  -> DMA h^T tile straight to DRAM out^T.
"""

import os

import numpy as np

import concourse.bacc as bacc
import concourse.tile as tile
from concourse import mybir

B, S, D = 8, 4096, 1024
N_CORES = 8
P = 128  # partitions
C = 512  # seq chunk
N_CHUNKS = S // C  # 8
N_DT = D // P  # 8 d-tiles (contraction)
N_KT = D // P  # 8 output channel tiles (hidden dim = D)

F32 = mybir.dt.float32
# fp32r: full-rate PE (1 cyc/row at N>=256) with TF32-class precision.
# Set MINGRU_MM_F32=1 to fall back to exact fp32 matmuls (4x slower PE).
MM_DT = F32 if os.environ.get("MINGRU_MM_F32") else mybir.dt.float32r

_COMPILED = {}


def _build():
    nc = bacc.Bacc(
        "TRN2", target_bir_lowering=False, debug=False, num_devices=N_CORES
    )
    xt_d = nc.dram_tensor("xt", [D, S], MM_DT, kind="ExternalInput").ap()
    w_d = nc.dram_tensor("w", [D, 2 * D], MM_DT, kind="ExternalInput").ap()
    out_d = nc.dram_tensor("outT", [D, S], F32, kind="ExternalOutput").ap()

    AL = mybir.AluOpType
    SIG = mybir.ActivationFunctionType.Sigmoid

    with tile.TileContext(nc) as tc:
        with (
            tc.tile_pool(name="wpool", bufs=1) as wpool,
            tc.tile_pool(name="xtp", bufs=2) as xt_pool,
            tc.tile_pool(name="pw", bufs=3) as pw_pool,
            tc.tile_pool(name="hp", bufs=2) as h_pool,
            tc.tile_pool(name="pshg", bufs=8, space="PSUM") as psum_hg,
        ):
            def load_x_chunk(s0, names):
                tiles = []
                for j in range(N_DT):
                    t = xt_pool.tile(
                        [P, C],
                        MM_DT,
                        tag=f"xt{j}",
                        name=names[j] if names else None,
                    )
                    nc.sync.dma_start(t[:], xt_d[j * P : (j + 1) * P, s0 : s0 + C])
                    tiles.append(t)
                return tiles

            # PE warm-up: the HAM clock gate keeps the PE at 1.2 GHz until
            # it has been busy ~3.4us. Run throwaway matmuls on a zeroed
            # tile while the first DMAs land so the real matmuls start at
            # 2.4 GHz. (50 MMs end ~14us, just before the first real MM.)
            zwarm = pw_pool.tile([P, C], F32, tag="warm")
            nc.gpsimd.memset(zwarm[:], 0.0)
            pwarm = psum_hg.tile([P, C], F32, tag="ph")
            N_WARM = 14  # fp32 MMs: ~2 cold @1.7us + 12 warm @0.85us ~= 14us
            for r in range(N_WARM):
                nc.tensor.matmul(
                    pwarm[:],
                    zwarm[:, 0:P],
                    zwarm[:],
                    start=(r == 0),
                    stop=(r == N_WARM - 1),
                )

            # chunk 0 of x^T before W so the PE can start almost immediately
            x0 = load_x_chunk(0, [f"x0_{j}" for j in range(N_DT)])

            w_big = [
                wpool.tile([P, 2 * D], MM_DT, tag=f"w{j}", name=f"w_big{j}")
                for j in range(N_DT)
            ]

            def wload(k0, k1):
                # one DMA per j: 3D AP covering the hidden cols [k0*P, k1*P)
                # and the matching gate cols D + [k0*P, k1*P) together
                for j in range(N_DT):
                    dst = w_big[j].rearrange("p (b c) -> p b c", b=2)
                    src = w_d[j * P : (j + 1) * P, :].rearrange(
                        "r (b c) -> r b c", b=2
                    )
                    nc.sync.dma_start(
                        dst[:, :, k0 * P : k1 * P], src[:, :, k0 * P : k1 * P]
                    )

            # staged so the first matmul pair only waits for ~1MB of W;
            # chunk 1's x^T is prefetched right after W so it is resident
            # long before chunk 0's matmuls finish.
            wload(0, 1)
            wload(1, 4)
            wload(4, 8)
            x1 = load_x_chunk(C, [f"x1_{j}" for j in range(N_DT)])
            w_sb = [
                [w_big[j][:, kk * P : (kk + 1) * P] for j in range(N_DT)]
                for kk in range(2 * N_KT)
            ]

            prev_h = [None] * N_KT
            for sc in range(N_CHUNKS):
                s0 = sc * C
                # ---- load x^T chunk tiles [128d, C]
                if sc == 0:
                    xts = x0
                elif sc == 1:
                    xts = x1
                else:
                    xts = load_x_chunk(s0, None)
                # ---- per channel-tile k: matmuls + pointwise + scan + store
                for k in range(N_KT):
                    ph = psum_hg.tile([P, C], F32, tag="ph")  # hidden
                    for j in range(N_DT):
                        nc.tensor.matmul(
                            ph[:],
                            w_sb[k][j],
                            xts[j][:],
                            start=(j == 0),
                            stop=(j == N_DT - 1),
                        )
                    pg = psum_hg.tile([P, C], F32, tag="ph")  # gate
                    for j in range(N_DT):
                        nc.tensor.matmul(
                            pg[:],
                            w_sb[N_KT + k][j],
                            xts[j][:],
                            start=(j == 0),
                            stop=(j == N_DT - 1),
                        )
                    # a = sigmoid(-gate) = 1 - z
                    a_t = pw_pool.tile([P, C], F32, tag="a")
                    nc.scalar.activation(a_t[:], pg[:], SIG, scale=-1.0)
                    # sigh = sigmoid(hidden)
                    sigh = pw_pool.tile([P, C], F32, tag="sigh")
                    nc.scalar.activation(sigh[:], ph[:], SIG)
                    # g(hidden) = max(hidden + 0.5, sigmoid(hidden))
                    gh = pw_pool.tile([P, C], F32, tag="gh")
                    nc.vector.scalar_tensor_tensor(
                        gh[:], ph[:], 0.5, sigh[:], op0=AL.add, op1=AL.max
                    )
                    # bneg = (a - 1) * g = -(z * g)
                    bneg = pw_pool.tile([P, C], F32, tag="bneg")
                    nc.vector.scalar_tensor_tensor(
                        bneg[:], a_t[:], 1.0, gh[:], op0=AL.subtract, op1=AL.mult
                    )
                    # h_t = a_t * h_{t-1} - bneg_t  (linear recurrence)
                    h = h_pool.tile([P, C], F32, tag=f"h{k}")
                    init = 0.0 if prev_h[k] is None else prev_h[k][:, C - 1 : C]
                    if sc == N_CHUNKS - 1 and k >= N_KT - 2:
                        # kernel tail: split the final scans/stores in half so
                        # the store of the first half overlaps the second scan
                        H = C // 2
                        nc.vector.tensor_tensor_scan(
                            h[:, 0:H], a_t[:, 0:H], bneg[:, 0:H], init,
                            op0=AL.mult, op1=AL.subtract,
                        )
                        nc.sync.dma_start(
                            out_d[k * P : (k + 1) * P, s0 : s0 + H], h[:, 0:H]
                        )
                        nc.vector.tensor_tensor_scan(
                            h[:, H:C], a_t[:, H:C], bneg[:, H:C],
                            h[:, H - 1 : H], op0=AL.mult, op1=AL.subtract,
                        )
                        nc.sync.dma_start(
                            out_d[k * P : (k + 1) * P, s0 + H : s0 + C],
                            h[:, H:C],
                        )
                    else:
                        nc.vector.tensor_tensor_scan(
                            h[:], a_t[:], bneg[:], init,
                            op0=AL.mult, op1=AL.subtract,
                        )
                        nc.sync.dma_start(
                            out_d[k * P : (k + 1) * P, s0 : s0 + C], h[:]
                        )
                    prev_h[k] = h
    nc.compile()
    return nc


def _get_nc():
    key = str(MM_DT)
    if key not in _COMPILED:
        _COMPILED[key] = _build()
    return _COMPILED[key]


def kernel(x: np.ndarray, W_hg: np.ndarray) -> np.ndarray:
    from concourse.bass_utils import run_bass_kernel_spmd

    assert x.shape == (B, S, D) and W_hg.shape == (D, 2 * D)
    nc = _get_nc()
    x = np.asarray(x, dtype=np.float32)
    w = np.ascontiguousarray(W_hg, dtype=np.float32)
    in_maps = [
        {"xt": np.ascontiguousarray(x[b].T), "w": w} for b in range(N_CORES)
    ]
    res = run_bass_kernel_spmd(nc, in_maps, list(range(N_CORES)))
    out = np.empty((B, S, D), dtype=np.float32)
    for b in range(N_CORES):
        out[b] = res.results[b]["outT"].T
    return out


# revision 34
# speedup vs baseline: 1.2033x; 1.2033x over previous
"""MinGRU (parallel log-space scan) Trainium2 Bass kernel.

Problem (hardcoded):
    x:    [B=8, S=4096, D=1024] f32
    W_hg: [D=1024, 2*D=2048]    f32
    out:  [B=8, S=4096, D=1024] f32

    hg = x @ W_hg ; hidden, gate = split(hg)
    h_t = (1-z_t) * h_{t-1} + z_t * g(hidden_t),  z = sigmoid(gate),
    g(v) = v + 0.5 if v >= 0 else sigmoid(v)  ==  max(v + 0.5, sigmoid(v))

Sharding: data-parallel over batch, one batch row per NeuronCore (8 cores),
W_hg replicated.

Layout strategy: the scan must run along the free dimension (channels on
partitions), so the device works entirely in the transposed layout
hg^T/h^T = [channels, seq]. The host passes x pre-transposed per batch row
and transposes the returned h^T back, so the device does no layout
conversion at all — the PE runs only the projection matmuls (fp32r,
full rate), ACT runs the sigmoids, and the DVE runs the fused pointwise
ops plus the native tensor_tensor_scan linear recurrence.

Per-core pipeline over seq chunks of C=512:
  DMA x^T chunk tiles [128d, C]
  -> fp32r matmuls hg^T[k] = sum_j W[j,k]^T x^T[j] accumulated in PSUM
  -> ACT: a = sigmoid(-gate), sigh = sigmoid(hidden)      [PSUM -> SBUF]
  -> DVE: gh = (hidden + 0.5) max sigh ; bneg = (a - 1) * gh
  -> DVE: h = scan(a * h_prev) - bneg   (carry chained across chunks)
  -> DMA h^T tile straight to DRAM out^T.
"""

import os

import numpy as np

import concourse.bacc as bacc
import concourse.tile as tile
from concourse import mybir

B, S, D = 8, 4096, 1024
N_CORES = 8
P = 128  # partitions
C = 512  # seq chunk
N_CHUNKS = S // C  # 8
N_DT = D // P  # 8 d-tiles (contraction)
N_KT = D // P  # 8 output channel tiles (hidden dim = D)

F32 = mybir.dt.float32
# fp32r: full-rate PE (1 cyc/row at N>=256) with TF32-class precision.
# Set MINGRU_MM_F32=1 to fall back to exact fp32 matmuls (4x slower PE).
MM_DT = F32 if os.environ.get("MINGRU_MM_F32") else mybir.dt.float32r

_COMPILED = {}


def _build():
    nc = bacc.Bacc(
        "TRN2", target_bir_lowering=False, debug=False, num_devices=N_CORES
    )
    xt_d = nc.dram_tensor("xt", [D, S], MM_DT, kind="ExternalInput").ap()
    w_d = nc.dram_tensor("w", [D, 2 * D], MM_DT, kind="ExternalInput").ap()
    out_d = nc.dram_tensor("outT", [D, S], F32, kind="ExternalOutput").ap()

    AL = mybir.AluOpType
    SIG = mybir.ActivationFunctionType.Sigmoid

    with tile.TileContext(nc) as tc:
        with (
            tc.tile_pool(name="wpool", bufs=1) as wpool,
            tc.tile_pool(name="xtp", bufs=2) as xt_pool,
            tc.tile_pool(name="pw", bufs=3) as pw_pool,
            tc.tile_pool(name="hp", bufs=2) as h_pool,
            tc.tile_pool(name="pshg", bufs=8, space="PSUM") as psum_hg,
        ):
            def load_x_chunk(s0, names):
                tiles = []
                for j in range(N_DT):
                    t = xt_pool.tile(
                        [P, C],
                        MM_DT,
                        tag=f"xt{j}",
                        name=names[j] if names else None,
                    )
                    nc.sync.dma_start(t[:], xt_d[j * P : (j + 1) * P, s0 : s0 + C])
                    tiles.append(t)
                return tiles

            # chunk 0 of x^T before W so the PE can start almost immediately
            x0 = load_x_chunk(0, [f"x0_{j}" for j in range(N_DT)])

            w_big = [
                wpool.tile([P, 2 * D], MM_DT, tag=f"w{j}", name=f"w_big{j}")
                for j in range(N_DT)
            ]

            def wload(k0, k1):
                # one DMA per j: 3D AP covering the hidden cols [k0*P, k1*P)
                # and the matching gate cols D + [k0*P, k1*P) together
                for j in range(N_DT):
                    dst = w_big[j].rearrange("p (b c) -> p b c", b=2)
                    src = w_d[j * P : (j + 1) * P, :].rearrange(
                        "r (b c) -> r b c", b=2
                    )
                    nc.sync.dma_start(
                        dst[:, :, k0 * P : k1 * P], src[:, :, k0 * P : k1 * P]
                    )

            # staged so the first matmul pair only waits for ~1MB of W;
            # chunk 1's x^T is prefetched right after W so it is resident
            # long before chunk 0's matmuls finish.
            wload(0, 1)
            wload(1, 4)
            wload(4, 8)
            x1 = load_x_chunk(C, [f"x1_{j}" for j in range(N_DT)])
            w_sb = [
                [w_big[j][:, kk * P : (kk + 1) * P] for j in range(N_DT)]
                for kk in range(2 * N_KT)
            ]

            prev_h = [None] * N_KT
            for sc in range(N_CHUNKS):
                s0 = sc * C
                # ---- load x^T chunk tiles [128d, C]
                if sc == 0:
                    xts = x0
                elif sc == 1:
                    xts = x1
                else:
                    xts = load_x_chunk(s0, None)
                # ---- per channel-tile k: matmuls + pointwise + scan + store
                for k in range(N_KT):
                    ph = psum_hg.tile([P, C], F32, tag="ph")  # hidden
                    for j in range(N_DT):
                        nc.tensor.matmul(
                            ph[:],
                            w_sb[k][j],
                            xts[j][:],
                            start=(j == 0),
                            stop=(j == N_DT - 1),
                        )
                    pg = psum_hg.tile([P, C], F32, tag="ph")  # gate
                    for j in range(N_DT):
                        nc.tensor.matmul(
                            pg[:],
                            w_sb[N_KT + k][j],
                            xts[j][:],
                            start=(j == 0),
                            stop=(j == N_DT - 1),
                        )
                    # a = sigmoid(-gate) = 1 - z
                    a_t = pw_pool.tile([P, C], F32, tag="a")
                    nc.scalar.activation(a_t[:], pg[:], SIG, scale=-1.0)
                    # sigh = sigmoid(hidden)
                    sigh = pw_pool.tile([P, C], F32, tag="sigh")
                    nc.scalar.activation(sigh[:], ph[:], SIG)
                    # g(hidden) = max(hidden + 0.5, sigmoid(hidden))
                    gh = pw_pool.tile([P, C], F32, tag="gh")
                    nc.vector.scalar_tensor_tensor(
                        gh[:], ph[:], 0.5, sigh[:], op0=AL.add, op1=AL.max
                    )
                    # bneg = (a - 1) * g = -(z * g)
                    bneg = pw_pool.tile([P, C], F32, tag="bneg")
                    nc.vector.scalar_tensor_tensor(
                        bneg[:], a_t[:], 1.0, gh[:], op0=AL.subtract, op1=AL.mult
                    )
                    # h_t = a_t * h_{t-1} - bneg_t  (linear recurrence)
                    h = h_pool.tile([P, C], F32, tag=f"h{k}")
                    init = 0.0 if prev_h[k] is None else prev_h[k][:, C - 1 : C]
                    nc.vector.tensor_tensor_scan(
                        h[:], a_t[:], bneg[:], init, op0=AL.mult, op1=AL.subtract
                    )
                    prev_h[k] = h
                    nc.sync.dma_start(
                        out_d[k * P : (k + 1) * P, s0 : s0 + C], h[:]
                    )
    nc.compile()
    return nc


def _get_nc():
    key = str(MM_DT)
    if key not in _COMPILED:
        _COMPILED[key] = _build()
    return _COMPILED[key]


def kernel(x: np.ndarray, W_hg: np.ndarray) -> np.ndarray:
    from concourse.bass_utils import run_bass_kernel_spmd

    assert x.shape == (B, S, D) and W_hg.shape == (D, 2 * D)
    nc = _get_nc()
    x = np.asarray(x, dtype=np.float32)
    w = np.ascontiguousarray(W_hg, dtype=np.float32)
    in_maps = [
        {"xt": np.ascontiguousarray(x[b].T), "w": w} for b in range(N_CORES)
    ]
    res = run_bass_kernel_spmd(nc, in_maps, list(range(N_CORES)))
    out = np.empty((B, S, D), dtype=np.float32)
    for b in range(N_CORES):
        out[b] = res.results[b]["outT"].T
    return out
